# revision 1
# baseline (speedup 1.0000x reference)
"""Trainium2 Bass kernel for nn_Coarse_module_67345087201829.

Reference computes  out = sum_X rho_X . block_X  over three Kronecker-structured
(DIM x DIM) adjacency blocks (DIM = N*T = 6000):
    block_IT = kron(I_T, A)          (block diagonal: A at (t, t))
    block_CS = kron(C_T, I_S)        (I at (t, t'<t))
    block_CT = kron(C_T, A)          (A at (t, t'<t))
with per-row sigmoid gates rho_X (computed from  block_X @ features  + interven
terms).  So output block (t, t') is
    t' == t : diag(rho_IT[t-rows]) @ A
    t' <  t : diag(rho_CT[t-rows]) @ A + diag(rho_CS[t-rows])
    t' >  t : 0
The heavy work is writing the 144 MB dense output; everything else is tiny.

Sharding: the node axis is split across the 8 cores (padded 500 -> 512 = 8*64).
Each core handles its 64 nodes for all 12 time blocks (768 output rows), so the
SPMD program is identical across cores.  Time blocks are processed in pairs
(2k, 2k+1) stacked on 128 SBUF partitions so output DMAs fan across all 16 DMA
engines.  The gates are computed on-device: a (A_rows @ V) matmul on the tensor
engine (V's 24 columns carry the per-t feature projections for the IT and CT
gates), bias add + sigmoid, then per-partition-scalar scaling of the A /
identity row slabs on the vector engine.  Zero blocks (t' > t) are never
written: ExternalOutput DRAM is pre-zeroed by the runtime.  The default
program is the hand-scheduled Bacc pipeline (_build_program_raw); set
KERNEL_TILE=1 for the Tile-framework variant.
"""

import numpy as np

N = 500          # nodes
T = 12           # timestamps
F = 3
DIM = N * T      # 6000
NCORES = 8
NPC = 64         # nodes per core (padded: 8*64 = 512; SBUF partition slices
                 # must start at multiples of 32, and 128 partitions = full
                 # DMA port width)
NPAD = NCORES * NPC
P2 = 2 * NPC     # 128 partitions = two t-halves
KPAD = 512       # padded contraction dim for the A @ V matmul
NPAIR = T // 2   # 6 time-block pairs

_PROGRAM_CACHE = {}


def _build_program():
    import concourse.bacc as bacc
    import concourse.mybir as mybir
    import concourse.tile as tile

    dt = mybir.dt.float32
    AF = mybir.ActivationFunctionType
    OP = mybir.AluOpType

    # Bacc (not raw Bass): its compile() legalizes sync waits — TRN2 allows
    # at most one wait per instruction, extra waits become event semaphores.
    nc = bacc.Bacc("TRN2", target_bir_lowering=False, debug=False,
                   num_devices=NCORES)

    # packed inputs: fewer DMAs -> fewer triggers + semaphores
    ai = nc.dram_tensor("ai", [P2, 2 * N], dt, kind="ExternalInput").ap()
    # packed [A_rows^T | V]: one DMA -> one semaphore, because the lowered
    # Matmult/LoadWeights instruction supports only a single sync wait
    av = nc.dram_tensor("av", [KPAD, P2 + 2 * T], dt, kind="ExternalInput").ap()
    dd = nc.dram_tensor("dd", [P2, 3 * T], dt, kind="ExternalInput").ap()
    out = nc.dram_tensor("out", [T * NPC, DIM], dt, kind="ExternalOutput").ap()

    with tile.TileContext(nc) as tc:
        with (
            tc.tile_pool(name="const", bufs=1) as cpool,
            tc.tile_pool(name="psum", bufs=1, space="PSUM") as ppool,
            tc.tile_pool(name="gate", bufs=1) as gpool,
            tc.tile_pool(name="slab", bufs=6) as spool,
        ):
            ai_sb = cpool.tile([P2, 2 * N], dt)
            nc.sync.dma_start(out=ai_sb[:], in_=ai[:])
            a2_sb = ai_sb[:, 0:N]
            i2_sb = ai_sb[:, N:2 * N]
            # contraction dim on partitions: row j = c*128 + p  ->  [p, c, m]
            av_sb = cpool.tile([128, 4, P2 + 2 * T], dt)
            nc.gpsimd.dma_start(out=av_sb[:],
                                in_=av.rearrange("(c p) m -> p c m", p=128))
            dd_sb = cpool.tile([P2, 3 * T], dt)
            nc.sync.dma_start(out=dd_sb[:], in_=dd[:])
            dit_sb = dd_sb[:, 0:T]
            dcs_sb = dd_sb[:, T:2 * T]
            dct_sb = dd_sb[:, 2 * T:3 * T]

            # q[p, c] = sum_j A[node_p, j] * V[j, c]  (c: 0..11 IT, 12..23 CT)
            q_ps = ppool.tile([P2, 2 * T], dt)
            for c in range(4):
                nc.tensor.matmul(q_ps[:], av_sb[:, c, 0:P2],
                                 av_sb[:, c, P2:P2 + 2 * T],
                                 start=(c == 0), stop=(c == 3))

            # rho_X = sigmoid(q_X + D_X)   (126, 12); CS has no matmul part
            z_it = gpool.tile([P2, T], dt)
            nc.vector.tensor_add(z_it[:], q_ps[:, 0:T], dit_sb[:])
            rho_it = gpool.tile([P2, T], dt)
            nc.scalar.activation(rho_it[:], z_it[:], AF.Sigmoid)
            z_ct = gpool.tile([P2, T], dt)
            nc.vector.tensor_add(z_ct[:], q_ps[:, T:2 * T], dct_sb[:])
            rho_ct = gpool.tile([P2, T], dt)
            nc.scalar.activation(rho_ct[:], z_ct[:], AF.Sigmoid)
            rho_cs = gpool.tile([P2, T], dt)
            nc.scalar.activation(rho_cs[:], dcs_sb[:], AF.Sigmoid)

            # Per-pair scalar columns: top half <- even t, bottom half <- odd t
            def pick(dst, src):
                ev = src[0:NPC, :].rearrange("p (k u) -> p k u", u=2)[:, :, 0]
                od = src[NPC:P2, :].rearrange("p (k u) -> p k u", u=2)[:, :, 1]
                nc.vector.tensor_copy(dst[0:NPC, :], ev)
                nc.vector.tensor_copy(dst[NPC:P2, :], od)

            R_it = gpool.tile([P2, NPAIR], dt)
            pick(R_it, rho_it)
            R_ct = gpool.tile([P2, NPAIR], dt)
            pick(R_ct, rho_ct)
            R_cs = gpool.tile([P2, NPAIR], dt)
            pick(R_cs, rho_cs)

            # Per pair k (rows = t=2k on partitions 0:64, t=2k+1 on 64:128):
            #   blocks t' < 2k      : tct = rho_CT*A + rho_CS*I   (full width)
            #   block  t' = 2k      : top = rho_IT*A (U), bottom = tct
            #   block  t' = 2k+1    : top = 0 (pre-zeroed), bottom = rho_IT*A
            #   blocks t' > 2k+1    : 0 (pre-zeroed, never written)
            # Largest pairs first so the big broadcast transfers start
            # draining while the rest compute.
            for k in range(NPAIR - 1, -1, -1):
                r0 = 2 * k * NPC
                rows = slice(r0, r0 + P2)
                half = slice(NPC, P2) if k == 0 else slice(0, P2)
                # p2i + tct chained on the vector engine (no cross-engine dep)
                p2i = spool.tile([P2, N], dt, tag="p2i")
                nc.vector.tensor_scalar_mul(p2i[half, :], i2_sb[half, :],
                                            R_cs[half, k:k + 1])
                tct = spool.tile([P2, N], dt, tag="tct")
                nc.vector.scalar_tensor_tensor(
                    tct[half, :], in0=a2_sb[half, :],
                    scalar=R_ct[half, k:k + 1],
                    in1=p2i[half, :], op0=OP.mult, op1=OP.add)
                # rho_IT product on the (otherwise idle) scalar engine
                u = spool.tile([P2, N], dt, tag="u")
                nc.scalar.activation(u[:], a2_sb[:], AF.Copy,
                                     bias=0.0, scale=R_it[:, k:k + 1])
                if k > 0:
                    # one broadcast-source DMA covers all 2k repeated blocks:
                    # destination columns are contiguous
                    dest = out[rows, 0:2 * k * N].rearrange(
                        "p (b c) -> p b c", c=N)
                    src = tct[:, None, :].broadcast_to([P2, 2 * k, N])
                    nc.sync.dma_start(out=dest, in_=src)
                # U top -> diagonal block 2k, U bottom -> diagonal block 2k+1.
                # (A skewed single-DMA destination fans out to only 2 DMA
                # engines — 10x slower. Two half-width DMAs fan to 8 each.)
                nc.gpsimd.dma_start(
                    out=out[r0:r0 + NPC, 2 * k * N:(2 * k + 1) * N],
                    in_=u[0:NPC, :])
                nc.gpsimd.dma_start(
                    out=out[r0 + NPC:r0 + P2, (2 * k + 1) * N:(2 * k + 2) * N],
                    in_=u[NPC:P2, :])
                # tct bottom half -> block 2k bottom (diagonal-straddling)
                nc.sync.dma_start(
                    out=out[r0 + NPC:r0 + P2, 2 * k * N:(2 * k + 1) * N],
                    in_=tct[NPC:P2, :])
    nc.compile()
    return nc


def _build_program_raw():
    """Hand-scheduled Bacc version: ~6 semaphores, no Tile barrier overhead.

    Engine roles: sync triggers every DMA on the one in-order HWDGE queue
    (kept full -> 16 DMA engines never starve); PE does the A@V matvec; DVE
    does all elementwise tile scaling; ACT does the one sigmoid.  Each pair
    gets dedicated tct/u tiles so there is no buffer-reuse hazard at all.
    """
    from contextlib import ExitStack

    import concourse.bacc as bacc
    import concourse.mybir as mybir

    dt = mybir.dt.float32
    AF = mybir.ActivationFunctionType
    OP = mybir.AluOpType

    nc = bacc.Bacc("TRN2", target_bir_lowering=False, debug=False,
                   enable_asserts=False, num_devices=NCORES)

    ai = nc.dram_tensor("ai", [P2, 2 * N], dt, kind="ExternalInput").ap()
    av = nc.dram_tensor("av", [KPAD, P2 + 2 * T], dt, kind="ExternalInput").ap()
    dd = nc.dram_tensor("dd", [P2, 3 * T], dt, kind="ExternalInput").ap()
    out = nc.dram_tensor("out", [T * NPC, DIM], dt, kind="ExternalOutput").ap()

    order = list(range(NPAIR - 1, -1, -1))   # big pairs first
    n_out = sum(4 if k > 0 else 3 for k in order)

    with ExitStack() as ctx:
        e = ctx.enter_context
        ai_sb = e(nc.sbuf_tensor("ai_sb", [P2, 2 * N], dt))
        av_sb = e(nc.sbuf_tensor("av_sb", [128, 4, P2 + 2 * T], dt))
        dd_sb = e(nc.sbuf_tensor("dd_sb", [P2, 3 * T], dt))
        z2_sb = e(nc.sbuf_tensor("z2_sb", [P2, 2 * T], dt))
        rho2_sb = e(nc.sbuf_tensor("rho2_sb", [P2, 2 * T], dt))
        rhocs_sb = e(nc.sbuf_tensor("rhocs_sb", [P2, T], dt))
        Rit_sb = e(nc.sbuf_tensor("Rit_sb", [P2, NPAIR], dt))
        Rcs_sb = e(nc.sbuf_tensor("Rcs_sb", [P2, NPAIR], dt))
        Rct_sb = e(nc.sbuf_tensor("Rct_sb", [P2, NPAIR], dt))
        p2i_sb = [e(nc.sbuf_tensor(f"p2i{i}_sb", [P2, N], dt))
                  for i in range(NPAIR)]
        tct_sb = [e(nc.sbuf_tensor(f"tct{i}_sb", [P2, N], dt))
                  for i in range(NPAIR)]
        u_sb = [e(nc.sbuf_tensor(f"u{i}_sb", [P2, N], dt))
                for i in range(NPAIR)]
        q_ps = e(nc.psum_tensor("q_ps", [P2, 2 * T], dt))
        s_in = e(nc.semaphore("s_in"))
        s_in2 = e(nc.semaphore("s_in2"))
        s_pe = e(nc.semaphore("s_pe"))
        s_cs = e(nc.semaphore("s_cs"))
        s_z = e(nc.semaphore("s_z"))
        s_act = e(nc.semaphore("s_act"))
        s_dve = e(nc.semaphore("s_dve"))
        s_out = e(nc.semaphore("s_out"))
        blk = e(nc.Block())

        a2_sb = ai_sb[:, 0:N]
        i2_sb = ai_sb[:, N:2 * N]

        def pick(copy_op, dst, src):
            """dst[:, k] <- src[:, 2k] (top half) / src[:, 2k+1] (bottom)."""
            ev = src[0:NPC, :].rearrange("p (k u) -> p k u", u=2)[:, :, 0]
            od = src[NPC:P2, :].rearrange("p (k u) -> p k u", u=2)[:, :, 1]
            copy_op(dst[0:NPC, :], ev)
            return copy_op(dst[NPC:P2, :], od)

        @blk.sync
        def _(sync):
            # ai/dd load in parallel on ACT's queue; av as one DMA (chunked
            # loads serialize on trigger issue and finish later)
            sync.dma_start(out=av_sb[:],
                           in_=av.rearrange("(c p) m -> p c m", p=128)
                           ).then_inc(s_in, 16)
            for idx, k in enumerate(order):
                sync.wait_ge(s_dve, idx + 1)
                r0 = 2 * k * NPC
                rows = slice(r0, r0 + P2)
                tct = tct_sb[k]
                if k > 0:
                    dest = out[rows, 0:2 * k * N].rearrange(
                        "p (b c) -> p b c", c=N)
                    src = tct[:, None, :].broadcast_to([P2, 2 * k, N])
                    sync.dma_start(out=dest, in_=src).then_inc(s_out, 16)
                sync.dma_start(
                    out=out[r0 + NPC:r0 + P2, 2 * k * N:(2 * k + 1) * N],
                    in_=tct[NPC:P2, :]).then_inc(s_out, 16)
            sync.wait_ge(s_out, 16 * n_out)

        @blk.tensor
        def _(pe):
            pe.wait_ge(s_in, 16)
            for c in range(4):
                nc.tensor.matmul(q_ps[:], av_sb[:, c, 0:P2],
                                 av_sb[:, c, P2:P2 + 2 * T],
                                 start=(c == 0), stop=(c == 3))
            nc.tensor.drain().then_inc(s_pe, 1)

        @blk.scalar
        def _(act):
            # critical loads on ACT's own HWDGE queue, in parallel with
            # sync's av chunks; s_in2: ai -> 16, dd -> 32
            nc.scalar.dma_start(out=ai_sb[:], in_=ai[:]).then_inc(s_in2, 16)
            nc.scalar.dma_start(out=dd_sb[:], in_=dd[:]).then_inc(s_in2, 16)
            # CS gate has no matmul part: sigmoid as soon as dd lands
            act.wait_ge(s_in2, 32)
            nc.scalar.activation(rhocs_sb[:], dd_sb[:, T:2 * T],
                                 AF.Sigmoid)
            nc.scalar.drain().then_inc(s_cs, 1)
            act.wait_ge(s_z, 1)
            nc.scalar.activation(rho2_sb[:], z2_sb[:],
                                 AF.Sigmoid)
            nc.scalar.drain().then_inc(s_act, 1)
            # rho_IT per-pair scalars + U products + U write triggers all on
            # ACT, fully off the sync/DVE critical path

            def act_copy(dst, src):
                return nc.scalar.activation(dst, src, AF.Copy)

            pick(act_copy, Rit_sb[:], rho2_sb[:, 0:T])
            for k in order:
                r0 = 2 * k * NPC
                u = u_sb[k]
                nc.scalar.activation(u[:], a2_sb[:], AF.Copy, bias=0.0,
                                     scale=Rit_sb[:, k:k + 1])
                nc.scalar.drain()
                nc.scalar.dma_start(
                    out=out[r0:r0 + NPC, 2 * k * N:(2 * k + 1) * N],
                    in_=u[0:NPC, :]).then_inc(s_out, 16)
                nc.scalar.dma_start(
                    out=out[r0 + NPC:r0 + P2,
                            (2 * k + 1) * N:(2 * k + 2) * N],
                    in_=u[NPC:P2, :]).then_inc(s_out, 16)

        @blk.vector
        def _(dve):
            # while PE is still accumulating: rho_CS scalars + the first two
            # pairs' identity products (s_cs transitively implies ai+dd done)
            dve.wait_ge(s_cs, 1)
            pick(nc.vector.tensor_copy, Rcs_sb[:], rhocs_sb[:])
            for k in order[:2]:
                nc.vector.tensor_scalar_mul(p2i_sb[k][:], i2_sb[:],
                                            Rcs_sb[:, k:k + 1])
            # z-adds gate the IT/CT sigmoid, which gates everything else
            dve.wait_ge(s_pe, 1)
            ddp = dd_sb[:]
            q = q_ps[:]
            nc.vector.tensor_add(z2_sb[:, 0:T], q[:, 0:T], ddp[:, 0:T])
            nc.vector.tensor_add(z2_sb[:, T:2 * T], q[:, T:2 * T],
                                 ddp[:, 2 * T:3 * T])
            nc.vector.drain().then_inc(s_z, 1)
            dve.wait_ge(s_act, 1)
            pick(nc.vector.tensor_copy, Rct_sb[:], rho2_sb[:, T:2 * T])
            # interleave p2i/tct per pair so s_dve fires as early as possible
            for idx, k in enumerate(order):
                half = slice(NPC, P2) if k == 0 else slice(0, P2)
                if idx >= 2:
                    nc.vector.tensor_scalar_mul(p2i_sb[k][half, :],
                                                i2_sb[half, :],
                                                Rcs_sb[half, k:k + 1])
                nc.vector.scalar_tensor_tensor(
                    tct_sb[k][half, :], in0=a2_sb[half, :],
                    scalar=Rct_sb[half, k:k + 1],
                    in1=p2i_sb[k][half, :], op0=OP.mult, op1=OP.add)
                nc.vector.drain().then_inc(s_dve, 1)

    nc.compile()
    return nc


def _host_prep(his_raw_features, interven, adj,
               w1_IT, w2_IT, gw_IT, gb_IT,
               w1_CS, w2_CS, gw_CS, gb_CS,
               w1_CT, w2_CT, gw_CT, gb_CT):
    """Build the per-core input maps (all tiny; sharding + gate-bias prep)."""
    f32 = np.float32
    his = np.asarray(his_raw_features, f32)      # (T, N, F)
    itv = np.asarray(interven, f32)              # (T, N)
    A = np.asarray(adj, f32)                     # (N, N)

    # cur / cum selection, replicating the reference's f32-exact comparisons
    sA = float(np.asarray(adj, np.float64).sum())
    judge = sA * T
    cur = itv
    cum = (np.cumsum(itv.astype(np.float64), axis=0) - itv).astype(f32)
    bs = {"IT": T * sA, "CS": N * T * (T - 1) / 2.0, "CT": sA * T * (T - 1) / 2.0}
    ia = {X: (cum if bs[X] > judge else cur) for X in ("IT", "CS", "CT")}

    def sc(x):
        return float(np.asarray(x).ravel()[0])

    params = {
        "IT": (sc(w1_IT), sc(w2_IT), np.asarray(gw_IT, f32).ravel(), sc(gb_IT)),
        "CS": (sc(w1_CS), sc(w2_CS), np.asarray(gw_CS, f32).ravel(), sc(gb_CS)),
        "CT": (sc(w1_CT), sc(w2_CT), np.asarray(gw_CT, f32).ravel(), sc(gb_CT)),
    }

    g = {X: np.einsum("tnf,f->tn", his, params[X][2], dtype=np.float64).astype(f32)
         for X in params}                         # g_X[t, n] = F_t[n] . gw_X
    pg = {X: (np.cumsum(g[X].astype(np.float64), axis=0) - g[X]).astype(f32)
          for X in params}                        # exclusive prefix over t

    D = {}
    for X in params:
        w1, w2, gw, gb = params[X]
        G = float(gw.sum())
        d = ia[X] * G + w2 * g[X] + gb            # (T, N)
        if X == "CS":
            d = d + w1 * pg["CS"]                 # CS gate has no matvec part
        D[X] = d.astype(f32)

    # V (KPAD, 24): columns 0..11 = w1_IT * g_IT[t], 12..23 = w1_CT * pg_CT[t]
    V = np.zeros((KPAD, 2 * T), f32)
    V[:N, 0:T] = params["IT"][0] * g["IT"].T
    V[:N, T:2 * T] = params["CT"][0] * pg["CT"].T

    A_pad = np.zeros((NPAD, N), f32)
    A_pad[:N] = A
    I_pad = np.zeros((NPAD, N), f32)
    I_pad[:N, :N] = np.eye(N, dtype=f32)
    D_pad = {X: np.zeros((T, NPAD), f32) for X in D}
    for X in D:
        D_pad[X][:, :N] = D[X]

    in_maps = []
    for c in range(NCORES):
        sl = slice(c * NPC, (c + 1) * NPC)
        a_sl = A_pad[sl]                          # (63, 500)
        a2_c = np.concatenate([a_sl, a_sl], axis=0)           # (126, 500)
        i_sl = I_pad[sl]
        i2_c = np.concatenate([i_sl, i_sl], axis=0)
        av_c = np.zeros((KPAD, P2 + 2 * T), f32)
        av_c[:N, :P2] = a2_c.T                    # lhsT: [j, p] = A[node_p, j]
        av_c[:, P2:] = V
        dmap = {}
        for X in D_pad:
            d_sl = D_pad[X][:, sl].T              # (NPC, 12)
            dmap[X] = np.concatenate([d_sl, d_sl], axis=0)    # (P2, 12)
        in_maps.append({
            "ai": np.concatenate([a2_c, i2_c], axis=1),
            "av": av_c,
            "dd": np.concatenate([dmap["IT"], dmap["CS"], dmap["CT"]], axis=1),
        })
    return in_maps


def _gather(results):
    final = np.zeros((T, N, DIM), np.float32)
    for c in range(NCORES):
        g0 = c * NPC
        g1 = min(g0 + NPC, N)
        if g1 <= g0:
            continue
        slab = results[c]["out"].reshape(T, NPC, DIM)
        final[:, g0:g1, :] = slab[:, : g1 - g0, :]
    return final.reshape(DIM, DIM)


def kernel(**inputs):
    import os

    from concourse.bass_utils import run_bass_kernel_spmd

    if "nc" not in _PROGRAM_CACHE:
        if os.environ.get("KERNEL_TILE") == "1":
            _PROGRAM_CACHE["nc"] = _build_program()
        else:
            _PROGRAM_CACHE["nc"] = _build_program_raw()
    nc = _PROGRAM_CACHE["nc"]

    in_maps = _host_prep(**inputs)
    res = run_bass_kernel_spmd(nc, in_maps, list(range(NCORES)))
    return _gather(res.results)

